# revision 23
# baseline (speedup 1.0000x reference)
"""Causal self-attention with RoPE (B=2, T=1024, C=2048, H=16) on 8 TRN2
NeuronCores, head-parallel tensor sharding (2 heads per core).

Hard-won TRN2 facts baked into this kernel (measured via in-NEFF repetition
slope timing; single-exec wall deltas under axon are unreliable):
  - PE matmuls whose written PSUM region is wider than 256 f32 columns AND
    whose rhs access pattern cannot collapse to a contiguous 1D run (any
    slice of a wider SBUF tile) hit a ~115us/instruction slow path. Fix:
    chunk every such matmul to <=256-wide pieces.
  - A PSUM accumulation group (start..stop) must keep a CONSTANT written
    region: shrinking-suffix groups (the textbook causal-attention trick)
    run ~100x slow; two interleaved groups in ONE bank corrupt results.
    Fix: separate PSUM banks per 256-wide accumulation stream, zero-fill
    the masked prefix of the exp() output so every accumulate covers the
    full region.
  - Each extra collective costs ~0.8ms in serialized rendezvous: stage
    replicated x / cos-sin tables through the (cheap) input blob instead of
    AllGathering them, and merge per-batch y gathers into ONE bf16
    AllGather.
  - All staged inputs ride in ONE flat bf16 blob, copied DRAM->DRAM (64KB
    descriptors) to internal mirrors, spread across the three DMA queues
    (SP-HWDGE / Act-HWDGE / Pool-SWDGE); SBUF loads read the fast mirrors.
Compute: bf16 QKV projections (contraction on partitions), RoPE via SBUF
partition-shift DMA + DVE FMA, causal attention in [tk, tq] layout with exp
on ScalarE straight from PSUM and the softmax denominator via an
all-ones-lhsT matmul, bf16 y AllGather, bf16 output projection. The output
is staged chunk-major bf16 and copied flat to the ExternalOutput as soon as
each chunk finishes. Host upcasts to f32 and reassembles.
"""
import numpy as np

import concourse.bass as bass
import concourse.mybir as mybir
import concourse.tile as tile
from concourse import bacc
from concourse.bass_utils import run_bass_kernel_spmd

F32 = mybir.dt.float32
F32R = mybir.dt.float32r
BF16 = mybir.dt.bfloat16

B, T, C = 2, 1024, 2048
H = 16
D = C // H            # 128
BT = B * T            # 2048
NCORES = 8
HL = H // NCORES      # heads per core = 2
CL = HL * D           # local channels = 256
ATT_SCALE = 1.0 / float(np.sqrt(D))
ROPE_BASE = 10000.0
NEG = -1.0e30

CT = C // 128         # 16 contraction tiles
TB = BT // 512        # 4 token blocks of 512
CSH = 2 * D // NCORES  # rows of the cos/sin shard per core = 32
RG = [list(range(NCORES))]

SEG = CL * BT          # 524288 elements per 1MB bf16 segment
XSEG = C * BT          # replicated full x^T (4M elements, 8MB bf16)
CSSEG = 2 * D * T      # full cos/sin table [256, 1024]
BLOB = XSEG + 4 * SEG + CSSEG  # flat input blob length (elements)
OUTCH = 128 * 1024     # output chunk: 128 partitions x 1024 cols


def _rope_tables():
    inv_freq = 1.0 / (ROPE_BASE ** (np.arange(0, D, 2, dtype=np.float64) / D))
    t = np.arange(T, dtype=np.float64)
    freqs = np.outer(t, inv_freq)                       # [T, D/2]
    emb = np.concatenate([freqs, freqs], axis=-1)        # [T, D]
    cos = np.cos(emb).astype(np.float32)                 # [T, D]
    sin = np.sin(emb).astype(np.float32)
    cosT = np.ascontiguousarray(cos.T)                   # [D, T]
    sinT = np.ascontiguousarray(sin.T)
    sgn_sinT = sinT.copy()
    sgn_sinT[: D // 2] *= -1.0                           # rotate_half sign
    return cosT, sgn_sinT


def _build(use_collective=True, reps=1):
    nc = bacc.Bacc("TRN2", target_bir_lowering=False, debug=False,
                   num_devices=NCORES)

    # direct staged inputs (EI reads are full-speed HBM on this runtime):
    # replicated x^T, the four weight shards side by side, cos/sin tables
    x_d = nc.dram_tensor("x", [C, BT], BF16, kind="ExternalInput").ap()
    w_d = nc.dram_tensor("w", [128, 4 * CT * CL], BF16,
                         kind="ExternalInput").ap()
    css_d = nc.dram_tensor("css", [2 * D, T], BF16,
                           kind="ExternalInput").ap()
    # output, chunk-major: chunk bh holds [128, 1024] contiguously
    out_d = nc.dram_tensor("out", [4, OUTCH], BF16,
                           kind="ExternalOutput").ap()

    yin_d = [nc.dram_tensor(f"yin{b}", [CL, T], BF16) for b in range(B)]
    yout_d = [nc.dram_tensor(f"yout{b}", [C, T], BF16, addr_space="Shared")
              for b in range(B)]

    with tile.TileContext(nc) as tc:
        with (
            tc.tile_pool(name="wpool", bufs=1) as wpool,
            tc.tile_pool(name="const", bufs=1) as cpool,
            tc.tile_pool(name="qkv", bufs=1) as qkvpool,
            tc.tile_pool(name="xs", bufs=6) as xspool,
            tc.tile_pool(name="rope", bufs=2) as ropepool,
            tc.tile_pool(name="att", bufs=3) as attpool,
            tc.tile_pool(name="wo", bufs=4) as wopool,
        ):
          for _rep in range(reps):
            # ---- weights / tables: SBUF loads straight from the EIs ----
            wq_sb = wpool.tile([128, CT * CL], BF16, tag="wq")
            wk_sb = wpool.tile([128, CT * CL], BF16, tag="wk")
            wv_sb = wpool.tile([128, CT * CL], BF16, tag="wv")
            wo_sb = wpool.tile([128, CT * CL], BF16, tag="wo")
            NW = CT * CL
            for i, (w_sb, eng) in enumerate(((wq_sb, nc.sync),
                                             (wk_sb, nc.scalar),
                                             (wv_sb, nc.sync),
                                             (wo_sb, nc.gpsimd))):
                eng.dma_start(out=w_sb[:],
                              in_=w_d[:, i * NW:(i + 1) * NW])

            cs_ld = cpool.tile([D, T], BF16, tag="cosld")
            sn_ld = cpool.tile([D, T], BF16, tag="sinld")
            nc.scalar.dma_start(out=cs_ld[:], in_=css_d[0:D, :])
            nc.sync.dma_start(out=sn_ld[:], in_=css_d[D:2 * D, :])
            cos_sb = cpool.tile([D, T], F32, tag="cos")
            sin_sb = cpool.tile([D, T], F32, tag="sin")
            nc.vector.tensor_copy(cos_sb[:], cs_ld[:])
            nc.vector.tensor_copy(sin_sb[:], sn_ld[:])

            ones_f = cpool.tile([128, 128], F32, tag="onesf")
            nc.gpsimd.memset(ones_f[:], 1.0)
            ones_sb = cpool.tile([128, 128], F32R, tag="ones")
            nc.vector.tensor_copy(ones_sb[:], ones_f[:])
            zer_f = cpool.tile([128, 384], F32, tag="zerf")
            nc.gpsimd.memset(zer_f[:], 0.0)
            zer_sb = cpool.tile([128, 384], F32R, tag="zer")
            nc.vector.tensor_copy(zer_sb[:], zer_f[:])

            # additive causal mask for diagonal 128x128 blocks:
            # rows=tk, cols=tq; keep (0.0) where tk <= tq else NEG
            mask_sb = cpool.tile([128, 128], F32, tag="mask")
            nc.gpsimd.memset(mask_sb[:], 0.0)
            nc.gpsimd.affine_select(
                out=mask_sb[:], in_=mask_sb[:],
                compare_op=mybir.AluOpType.is_ge,
                fill=NEG, base=0,
                pattern=[[1, 128]], channel_multiplier=-1,
            )

            # persistent qkv/y activations
            qT = [qkvpool.tile([D, BT], F32R, tag=f"qT{h}", name=f"qT{h}")
                  for h in range(HL)]
            kT = [qkvpool.tile([D, BT], F32R, tag=f"kT{h}", name=f"kT{h}")
                  for h in range(HL)]
            v_sb = qkvpool.tile([128, (BT // 128) * CL], F32R, tag="v")
            yT = [qkvpool.tile([D, BT], BF16, tag=f"yT{h}", name=f"yT{h}")
                  for h in range(HL)]

            # ---- phases 1+2 interleaved per batch: batch b attention and
            # its y AllGather overlap batch b+1 QKV projections ----
            for b in range(B):
              bcol = b * T
              if phases >= 1:
                with tc.tile_pool(name=f"psqkv{b}", bufs=1,
                                  space="PSUM") as psq:
                  for tb in (2 * b, 2 * b + 1):
                    tcol = tb * 512
                    ccol = tcol - b * T   # col into T-wide tables
                    ps_q = [psq.tile([128, 512], F32, tag=f"pq{h}", name=f"pq{h}")
                            for h in range(HL)]
                    ps_k = [psq.tile([128, 512], F32, tag=f"pk{h}", name=f"pk{h}")
                            for h in range(HL)]
                    ps_v = [psq.tile([128, CL], F32, tag=f"pv{i}", name=f"pv{i}")
                            for i in range(4)]
                    for ct in range(CT):
                        xs = xspool.tile([128, 512], BF16, tag="xs")
                        eng = nc.sync if ct % 2 == 0 else nc.scalar
                        eng.dma_start(
                            out=xs[:],
                            in_=x_d[ct * 128:(ct + 1) * 128,
                                    tcol:tcol + 512],
                        )
                        st, sp = ct == 0, ct == CT - 1
                        for h in range(HL):
                            nc.tensor.matmul(
                                ps_q[h][:],
                                wq_sb[:, ct * CL + h * D: ct * CL + (h + 1) * D],
                                xs[:], start=st, stop=sp)
                            nc.tensor.matmul(
                                ps_k[h][:],
                                wk_sb[:, ct * CL + h * D: ct * CL + (h + 1) * D],
                                xs[:], start=st, stop=sp)
                        for i in range(4):
                            nc.tensor.matmul(
                                ps_v[i][:],
                                xs[:, i * 128:(i + 1) * 128],
                                wv_sb[:, ct * CL:(ct + 1) * CL],
                                start=st, stop=sp)
                    # rope on q, k; plain copy for v
                    for h in range(HL):
                        for name, ps, dst in (("q", ps_q[h], qT[h]),
                                              ("k", ps_k[h], kT[h])):
                            tmp = ropepool.tile([128, 512], F32, tag="rtmp")
                            nc.vector.tensor_copy(tmp[:], ps[:])
                            rot = ropepool.tile([128, 512], F32, tag="rrot")
                            nc.gpsimd.dma_start(out=rot[0:64, :],
                                                in_=tmp[64:128, :])
                            nc.gpsimd.dma_start(out=rot[64:128, :],
                                                in_=tmp[0:64, :])
                            t1 = ropepool.tile([128, 512], F32, tag="rt1")
                            nc.vector.tensor_mul(
                                t1[:], ps[:], cos_sb[:, ccol:ccol + 512])
                            t2 = ropepool.tile([128, 512], F32, tag="rt2")
                            nc.gpsimd.tensor_mul(
                                t2[:], rot[:], sin_sb[:, ccol:ccol + 512])
                            nc.vector.tensor_add(
                                dst[:, tcol:tcol + 512], t1[:], t2[:])
                    for i in range(4):
                        gt = tb * 4 + i
                        nc.vector.tensor_copy(
                            v_sb[:, gt * CL:(gt + 1) * CL], ps_v[i][:])

              if phases >= 2:
                with tc.tile_pool(name=f"psatt{b}", bufs=1,
                                  space="PSUM") as psa:
                    ps_s2 = [psa.tile([128, 512], F32, tag=f"s{i}", name=f"s{i}")
                             for i in range(3)]
                    for h in range(HL):
                        for jj in range(2):
                            qcol = bcol + jj * 512
                            njt = 4 * jj + 4
                            ps_y = [psa.tile([128, 512], F32, tag=f"y{i}",
                                             name=f"psy{i}")
                                    for i in range(2)]
                            ps_l = [psa.tile([128, 512], F32, tag=f"l{i}",
                                             name=f"psl{i}")
                                    for i in range(2)]
                            for j in range(njt):
                                c0 = max(0, j * 128 - jj * 512)
                                ps_s = ps_s2[j % 3]
                                for cc in range(c0, 512, 256):
                                    cw = min(256, 512 - cc)
                                    nc.tensor.matmul(
                                        ps_s[:, cc:cc + cw],
                                        kT[h][:, bcol + j * 128: bcol + (j + 1) * 128],
                                        qT[h][:, qcol + cc: qcol + cc + cw],
                                        start=True, stop=True)
                                diag0 = j * 128 - jj * 512
                                if 0 <= diag0 < 512:
                                    nc.vector.tensor_add(
                                        ps_s[:, diag0:diag0 + 128],
                                        ps_s[:, diag0:diag0 + 128],
                                        mask_sb[:])
                                p = attpool.tile([128, 512], F32R, tag="p")
                                if c0 > 0:
                                    nc.vector.tensor_copy(
                                        p[:, 0:c0], zer_sb[:, 0:c0])
                                nc.scalar.activation(
                                    p[:, c0:512], ps_s[:, c0:512],
                                    mybir.ActivationFunctionType.Exp,
                                    scale=ATT_SCALE)
                                st, sp = j == 0, j == njt - 1
                                gt = (bcol // 128) + j
                                for ci in range(2):
                                    pc = p[:, ci * 256:(ci + 1) * 256]
                                    nc.tensor.matmul(
                                        ps_l[ci][:, 0:256], ones_sb[:],
                                        pc, start=st, stop=sp)
                                    nc.tensor.matmul(
                                        ps_y[ci][:, 0:256],
                                        v_sb[:, gt * CL + h * D: gt * CL + (h + 1) * D],
                                        pc, start=st, stop=sp)
                            rec = attpool.tile([128, 512], F32, tag="rec")
                            for ci in range(2):
                                nc.vector.reciprocal(
                                    rec[:, ci * 256:(ci + 1) * 256],
                                    ps_l[ci][:, 0:256])
                                nc.vector.tensor_mul(
                                    yT[h][:, qcol + ci * 256:
                                          qcol + (ci + 1) * 256],
                                    ps_y[ci][:, 0:256],
                                    rec[:, ci * 256:(ci + 1) * 256])
                # ship this batch's yT shard and AllGather it; the gather
                # overlaps the next batch's projections / output projection
                for h in range(HL):
                    eng = nc.sync if h % 2 == 0 else nc.scalar
                    eng.dma_start(
                        out=yin_d[b].ap()[h * D:(h + 1) * D, :],
                        in_=yT[h][:, bcol:bcol + T])
                if use_collective:
                    nc.gpsimd.collective_compute(
                        "AllGather", mybir.AluOpType.bypass,
                        replica_groups=RG,
                        ins=[yin_d[b].ap()],
                        outs=[yout_d[b].ap()],
                    )
                else:
                    nc.gpsimd.dma_start(out=yout_d[b].ap()[0:CL, :],
                                        in_=yin_d[b].ap())

            # ---- phase 3: output projection (this core's 256 columns) ----
            # each bh chunk: PSUM -> bf16 SBUF -> internal obuf (fast) ->
            # flat copy to the staged output, overlapping later chunks
            ob_big = qkvpool.tile([128, CT * CL], BF16, tag="obig")
            with tc.tile_pool(name="pso", bufs=1, space="PSUM") as pso:
                for b in range(B):
                    for half in range(2):
                        bh = b * 2 + half
                        hcol = half * 512
                        ps_o = [pso.tile([128, CL], F32, tag=f"po{i}", name=f"po{i}")
                                for i in range(4)]
                        for ct in range(CT):
                            yg = wopool.tile([128, 512], BF16, tag="yg")
                            eng = nc.sync if ct % 2 == 0 else nc.scalar
                            eng.dma_start(
                                out=yg[:],
                                in_=yout_d[b].ap()[ct * 128:(ct + 1) * 128,
                                                   hcol:hcol + 512])
                            st, sp = ct == 0, ct == CT - 1
                            for i in range(4):
                                nc.tensor.matmul(
                                    ps_o[i][:],
                                    yg[:, i * 128:(i + 1) * 128],
                                    wo_sb[:, ct * CL:(ct + 1) * CL],
                                    start=st, stop=sp)
                        for i in range(4):
                            nc.vector.tensor_copy(
                                ob_big[:, bh * 1024 + i * CL:
                                       bh * 1024 + (i + 1) * CL],
                                ps_o[i][:])
                        eng = nc.sync if bh % 2 == 0 else nc.scalar
                        eng.dma_start(
                            out=out_d[bh:bh + 1, :],
                            in_=ob_big[:, bh * 1024:(bh + 1) * 1024])

    nc.compile()
    return nc


_NC_CACHE = None


def _get_nc():
    global _NC_CACHE
    if _NC_CACHE is None:
        _NC_CACHE = _build()
    return _NC_CACHE


def make_in_maps(x, Wq, Wk, Wv, Wo):
    import ml_dtypes

    x = np.asarray(x, dtype=np.float32)
    xT = np.ascontiguousarray(x.reshape(BT, C).T)        # [C, BT]
    cosT, sinT = _rope_tables()
    csfull = np.concatenate([cosT, sinT], axis=0)        # [256, T]

    def conv(a):
        return np.ascontiguousarray(a).astype(ml_dtypes.bfloat16)

    def wlay(wT):
        # [C, CL] -> [128, CT*CL] with partition p holding WT[ct*128+p, :]
        return np.ascontiguousarray(
            wT.reshape(CT, 128, CL).transpose(1, 0, 2).reshape(128, CT * CL))

    xb = conv(xT)
    csb = conv(csfull)
    in_maps = []
    for m in range(NCORES):
        sl = slice(m * CL, (m + 1) * CL)
        wall = np.concatenate(
            [conv(wlay(np.asarray(W)[sl, :].T)) for W in (Wq, Wk, Wv, Wo)],
            axis=1)
        in_maps.append({"x": xb, "w": wall, "css": csb})
    return in_maps


def kernel(x, Wq, Wk, Wv, Wo, _trace=False):
    in_maps = make_in_maps(x, Wq, Wk, Wv, Wo)
    nc = _get_nc()
    res = run_bass_kernel_spmd(nc, in_maps, list(range(NCORES)),
                               trace=_trace)
    outs = []
    for m in range(NCORES):
        arr = np.asarray(res.results[m]["out"]).astype(np.float32)
        arr = arr.reshape(4, 128, 4, CL)                 # (bh, p, i, c)
        outs.append(arr.transpose(0, 2, 1, 3).reshape(BT, CL))
    out = np.ascontiguousarray(np.concatenate(outs, axis=1))
    out = out.reshape(B, T, C)
    if _trace:
        return out, res
    return out


# revision 24
# speedup vs baseline: 1.7145x; 1.7145x over previous
"""Causal self-attention with RoPE (B=2, T=1024, C=2048, H=16) on 8 TRN2
NeuronCores, head-parallel tensor sharding (2 heads per core).

Hard-won TRN2 facts baked into this kernel (measured via in-NEFF repetition
slope timing; single-exec wall deltas under axon are unreliable):
  - PE matmuls whose written PSUM region is wider than 256 f32 columns AND
    whose rhs access pattern cannot collapse to a contiguous 1D run (any
    slice of a wider SBUF tile) hit a ~115us/instruction slow path. Fix:
    chunk every such matmul to <=256-wide pieces.
  - A PSUM accumulation group (start..stop) must keep a CONSTANT written
    region: shrinking-suffix groups (the textbook causal-attention trick)
    run ~100x slow; two interleaved groups in ONE bank corrupt results.
    Fix: separate PSUM banks per 256-wide accumulation stream, zero-fill
    the masked prefix of the exp() output so every accumulate covers the
    full region.
  - Each extra collective costs ~0.8ms in serialized rendezvous: stage
    replicated x / cos-sin tables through the (cheap) input blob instead of
    AllGathering them, and merge per-batch y gathers into ONE bf16
    AllGather.
  - All staged inputs ride in ONE flat bf16 blob, copied DRAM->DRAM (64KB
    descriptors) to internal mirrors, spread across the three DMA queues
    (SP-HWDGE / Act-HWDGE / Pool-SWDGE); SBUF loads read the fast mirrors.
Compute: bf16 QKV projections (contraction on partitions), RoPE via SBUF
partition-shift DMA + DVE FMA, causal attention in [tk, tq] layout with exp
on ScalarE straight from PSUM and the softmax denominator via an
all-ones-lhsT matmul, bf16 y AllGather, bf16 output projection. The output
is staged chunk-major bf16 and copied flat to the ExternalOutput as soon as
each chunk finishes. Host upcasts to f32 and reassembles.
"""
import numpy as np

import concourse.bass as bass
import concourse.mybir as mybir
import concourse.tile as tile
from concourse import bacc
from concourse.bass_utils import run_bass_kernel_spmd

F32 = mybir.dt.float32
F32R = mybir.dt.float32r
BF16 = mybir.dt.bfloat16

B, T, C = 2, 1024, 2048
H = 16
D = C // H            # 128
BT = B * T            # 2048
NCORES = 8
HL = H // NCORES      # heads per core = 2
CL = HL * D           # local channels = 256
ATT_SCALE = 1.0 / float(np.sqrt(D))
ROPE_BASE = 10000.0
NEG = -1.0e30

CT = C // 128         # 16 contraction tiles
TB = BT // 512        # 4 token blocks of 512
CSH = 2 * D // NCORES  # rows of the cos/sin shard per core = 32
RG = [list(range(NCORES))]

SEG = CL * BT          # 524288 elements per 1MB bf16 segment
XSEG = C * BT          # replicated full x^T (4M elements, 8MB bf16)
CSSEG = 2 * D * T      # full cos/sin table [256, 1024]
BLOB = XSEG + 4 * SEG + CSSEG  # flat input blob length (elements)
OUTCH = 128 * 1024     # output chunk: 128 partitions x 1024 cols


def _rope_tables():
    inv_freq = 1.0 / (ROPE_BASE ** (np.arange(0, D, 2, dtype=np.float64) / D))
    t = np.arange(T, dtype=np.float64)
    freqs = np.outer(t, inv_freq)                       # [T, D/2]
    emb = np.concatenate([freqs, freqs], axis=-1)        # [T, D]
    cos = np.cos(emb).astype(np.float32)                 # [T, D]
    sin = np.sin(emb).astype(np.float32)
    cosT = np.ascontiguousarray(cos.T)                   # [D, T]
    sinT = np.ascontiguousarray(sin.T)
    sgn_sinT = sinT.copy()
    sgn_sinT[: D // 2] *= -1.0                           # rotate_half sign
    return cosT, sgn_sinT


def _build(use_collective=True, reps=1):
    nc = bacc.Bacc("TRN2", target_bir_lowering=False, debug=False,
                   num_devices=NCORES)

    # direct staged inputs (EI reads are full-speed HBM on this runtime):
    # replicated x^T, the four weight shards side by side, cos/sin tables
    x_d = nc.dram_tensor("x", [C, BT], BF16, kind="ExternalInput").ap()
    w_d = nc.dram_tensor("w", [128, 4 * CT * CL], BF16,
                         kind="ExternalInput").ap()
    css_d = nc.dram_tensor("css", [2 * D, T], BF16,
                           kind="ExternalInput").ap()
    # output, chunk-major: chunk bh holds [128, 1024] contiguously
    out_d = nc.dram_tensor("out", [4, OUTCH], BF16,
                           kind="ExternalOutput").ap()

    yin_d = nc.dram_tensor("yin", [CL, BT], BF16)
    yout_d = nc.dram_tensor("yout", [C, BT], BF16, addr_space="Shared")

    with tile.TileContext(nc) as tc:
        with (
            tc.tile_pool(name="wpool", bufs=1) as wpool,
            tc.tile_pool(name="const", bufs=1) as cpool,
            tc.tile_pool(name="qkv", bufs=1) as qkvpool,
            tc.tile_pool(name="xs", bufs=6) as xspool,
            tc.tile_pool(name="rope", bufs=2) as ropepool,
            tc.tile_pool(name="att", bufs=3) as attpool,
            tc.tile_pool(name="wo", bufs=4) as wopool,
        ):
          for _rep in range(reps):
            # ---- weights / tables: SBUF loads straight from the EIs ----
            wq_sb = wpool.tile([128, CT * CL], BF16, tag="wq")
            wk_sb = wpool.tile([128, CT * CL], BF16, tag="wk")
            wv_sb = wpool.tile([128, CT * CL], BF16, tag="wv")
            wo_sb = wpool.tile([128, CT * CL], BF16, tag="wo")
            NW = CT * CL
            for i, (w_sb, eng) in enumerate(((wq_sb, nc.sync),
                                             (wk_sb, nc.scalar),
                                             (wv_sb, nc.sync),
                                             (wo_sb, nc.gpsimd))):
                eng.dma_start(out=w_sb[:],
                              in_=w_d[:, i * NW:(i + 1) * NW])

            cs_ld = cpool.tile([D, T], BF16, tag="cosld")
            sn_ld = cpool.tile([D, T], BF16, tag="sinld")
            nc.scalar.dma_start(out=cs_ld[:], in_=css_d[0:D, :])
            nc.sync.dma_start(out=sn_ld[:], in_=css_d[D:2 * D, :])
            cos_sb = cpool.tile([D, T], F32, tag="cos")
            sin_sb = cpool.tile([D, T], F32, tag="sin")
            nc.vector.tensor_copy(cos_sb[:], cs_ld[:])
            nc.vector.tensor_copy(sin_sb[:], sn_ld[:])

            ones_f = cpool.tile([128, 128], F32, tag="onesf")
            nc.gpsimd.memset(ones_f[:], 1.0)
            ones_sb = cpool.tile([128, 128], F32R, tag="ones")
            nc.vector.tensor_copy(ones_sb[:], ones_f[:])
            zer_f = cpool.tile([128, 384], F32, tag="zerf")
            nc.gpsimd.memset(zer_f[:], 0.0)
            zer_sb = cpool.tile([128, 384], F32R, tag="zer")
            nc.vector.tensor_copy(zer_sb[:], zer_f[:])

            # additive causal mask for diagonal 128x128 blocks:
            # rows=tk, cols=tq; keep (0.0) where tk <= tq else NEG
            mask_sb = cpool.tile([128, 128], F32, tag="mask")
            nc.gpsimd.memset(mask_sb[:], 0.0)
            nc.gpsimd.affine_select(
                out=mask_sb[:], in_=mask_sb[:],
                compare_op=mybir.AluOpType.is_ge,
                fill=NEG, base=0,
                pattern=[[1, 128]], channel_multiplier=-1,
            )

            # persistent qkv/y activations
            qT = [qkvpool.tile([D, BT], F32R, tag=f"qT{h}", name=f"qT{h}")
                  for h in range(HL)]
            kT = [qkvpool.tile([D, BT], F32R, tag=f"kT{h}", name=f"kT{h}")
                  for h in range(HL)]
            v_sb = qkvpool.tile([128, (BT // 128) * CL], F32R, tag="v")
            yT = [qkvpool.tile([D, BT], BF16, tag=f"yT{h}", name=f"yT{h}")
                  for h in range(HL)]

            # ---- phases 1+2 interleaved per batch: batch b attention and
            # its y AllGather overlap batch b+1 QKV projections ----
            for b in range(B):
              bcol = b * T
              if phases >= 1:
                with tc.tile_pool(name=f"psqkv{b}", bufs=1,
                                  space="PSUM") as psq:
                  for tb in (2 * b, 2 * b + 1):
                    tcol = tb * 512
                    ccol = tcol - b * T   # col into T-wide tables
                    ps_q = [psq.tile([128, 512], F32, tag=f"pq{h}", name=f"pq{h}")
                            for h in range(HL)]
                    ps_k = [psq.tile([128, 512], F32, tag=f"pk{h}", name=f"pk{h}")
                            for h in range(HL)]
                    ps_v = [psq.tile([128, CL], F32, tag=f"pv{i}", name=f"pv{i}")
                            for i in range(4)]
                    for ct in range(CT):
                        xs = xspool.tile([128, 512], BF16, tag="xs")
                        eng = nc.sync if ct % 2 == 0 else nc.scalar
                        eng.dma_start(
                            out=xs[:],
                            in_=x_d[ct * 128:(ct + 1) * 128,
                                    tcol:tcol + 512],
                        )
                        st, sp = ct == 0, ct == CT - 1
                        for h in range(HL):
                            nc.tensor.matmul(
                                ps_q[h][:],
                                wq_sb[:, ct * CL + h * D: ct * CL + (h + 1) * D],
                                xs[:], start=st, stop=sp)
                            nc.tensor.matmul(
                                ps_k[h][:],
                                wk_sb[:, ct * CL + h * D: ct * CL + (h + 1) * D],
                                xs[:], start=st, stop=sp)
                        for i in range(4):
                            nc.tensor.matmul(
                                ps_v[i][:],
                                xs[:, i * 128:(i + 1) * 128],
                                wv_sb[:, ct * CL:(ct + 1) * CL],
                                start=st, stop=sp)
                    # rope on q, k; plain copy for v
                    for h in range(HL):
                        for name, ps, dst in (("q", ps_q[h], qT[h]),
                                              ("k", ps_k[h], kT[h])):
                            tmp = ropepool.tile([128, 512], F32, tag="rtmp")
                            nc.vector.tensor_copy(tmp[:], ps[:])
                            rot = ropepool.tile([128, 512], F32, tag="rrot")
                            nc.gpsimd.dma_start(out=rot[0:64, :],
                                                in_=tmp[64:128, :])
                            nc.gpsimd.dma_start(out=rot[64:128, :],
                                                in_=tmp[0:64, :])
                            t1 = ropepool.tile([128, 512], F32, tag="rt1")
                            nc.vector.tensor_mul(
                                t1[:], ps[:], cos_sb[:, ccol:ccol + 512])
                            t2 = ropepool.tile([128, 512], F32, tag="rt2")
                            nc.gpsimd.tensor_mul(
                                t2[:], rot[:], sin_sb[:, ccol:ccol + 512])
                            nc.vector.tensor_add(
                                dst[:, tcol:tcol + 512], t1[:], t2[:])
                    for i in range(4):
                        gt = tb * 4 + i
                        nc.vector.tensor_copy(
                            v_sb[:, gt * CL:(gt + 1) * CL], ps_v[i][:])

              if phases >= 2:
                with tc.tile_pool(name=f"psatt{b}", bufs=1,
                                  space="PSUM") as psa:
                    ps_s2 = [psa.tile([128, 512], F32, tag=f"s{i}", name=f"s{i}")
                             for i in range(3)]
                    for h in range(HL):
                        for jj in range(2):
                            qcol = bcol + jj * 512
                            njt = 4 * jj + 4
                            ps_y = [psa.tile([128, 512], F32, tag=f"y{i}",
                                             name=f"psy{i}")
                                    for i in range(2)]
                            ps_l = [psa.tile([128, 512], F32, tag=f"l{i}",
                                             name=f"psl{i}")
                                    for i in range(2)]
                            for j in range(njt):
                                c0 = max(0, j * 128 - jj * 512)
                                ps_s = ps_s2[j % 3]
                                for cc in range(c0, 512, 256):
                                    cw = min(256, 512 - cc)
                                    nc.tensor.matmul(
                                        ps_s[:, cc:cc + cw],
                                        kT[h][:, bcol + j * 128: bcol + (j + 1) * 128],
                                        qT[h][:, qcol + cc: qcol + cc + cw],
                                        start=True, stop=True)
                                diag0 = j * 128 - jj * 512
                                if 0 <= diag0 < 512:
                                    nc.vector.tensor_add(
                                        ps_s[:, diag0:diag0 + 128],
                                        ps_s[:, diag0:diag0 + 128],
                                        mask_sb[:])
                                p = attpool.tile([128, 512], F32R, tag="p")
                                if c0 > 0:
                                    nc.vector.tensor_copy(
                                        p[:, 0:c0], zer_sb[:, 0:c0])
                                nc.scalar.activation(
                                    p[:, c0:512], ps_s[:, c0:512],
                                    mybir.ActivationFunctionType.Exp,
                                    scale=ATT_SCALE)
                                st, sp = j == 0, j == njt - 1
                                gt = (bcol // 128) + j
                                for ci in range(2):
                                    pc = p[:, ci * 256:(ci + 1) * 256]
                                    nc.tensor.matmul(
                                        ps_l[ci][:, 0:256], ones_sb[:],
                                        pc, start=st, stop=sp)
                                    nc.tensor.matmul(
                                        ps_y[ci][:, 0:256],
                                        v_sb[:, gt * CL + h * D: gt * CL + (h + 1) * D],
                                        pc, start=st, stop=sp)
                            rec = attpool.tile([128, 512], F32, tag="rec")
                            for ci in range(2):
                                nc.vector.reciprocal(
                                    rec[:, ci * 256:(ci + 1) * 256],
                                    ps_l[ci][:, 0:256])
                                nc.vector.tensor_mul(
                                    yT[h][:, qcol + ci * 256:
                                          qcol + (ci + 1) * 256],
                                    ps_y[ci][:, 0:256],
                                    rec[:, ci * 256:(ci + 1) * 256])
                # ship this batch's yT shard now; ONE merged AllGather
                # after the last batch (extra collectives cost more than
                # the overlap they buy)
                for h in range(HL):
                    eng = nc.sync if h % 2 == 0 else nc.scalar
                    eng.dma_start(
                        out=yin_d.ap()[h * D:(h + 1) * D, bcol:bcol + T],
                        in_=yT[h][:, bcol:bcol + T])
                if b == B - 1:
                    if use_collective:
                        nc.gpsimd.collective_compute(
                            "AllGather", mybir.AluOpType.bypass,
                            replica_groups=RG,
                            ins=[yin_d.ap()],
                            outs=[yout_d.ap()],
                        )
                    else:
                        nc.gpsimd.dma_start(out=yout_d.ap()[0:CL, :],
                                            in_=yin_d.ap())

            # ---- phase 3: output projection (this core's 256 columns) ----
            # each bh chunk: PSUM -> bf16 SBUF -> internal obuf (fast) ->
            # flat copy to the staged output, overlapping later chunks
            ob_big = qkvpool.tile([128, CT * CL], BF16, tag="obig")
            with tc.tile_pool(name="pso", bufs=1, space="PSUM") as pso:
                for b in range(B):
                    for half in range(2):
                        bh = b * 2 + half
                        hcol = half * 512
                        ps_o = [pso.tile([128, CL], F32, tag=f"po{i}", name=f"po{i}")
                                for i in range(4)]
                        for ct in range(CT):
                            yg = wopool.tile([128, 512], BF16, tag="yg")
                            eng = nc.sync if ct % 2 == 0 else nc.scalar
                            eng.dma_start(
                                out=yg[:],
                                in_=yout_d.ap()[ct * 128:(ct + 1) * 128,
                                                b * T + hcol:
                                                b * T + hcol + 512])
                            st, sp = ct == 0, ct == CT - 1
                            for i in range(4):
                                nc.tensor.matmul(
                                    ps_o[i][:],
                                    yg[:, i * 128:(i + 1) * 128],
                                    wo_sb[:, ct * CL:(ct + 1) * CL],
                                    start=st, stop=sp)
                        for i in range(4):
                            nc.vector.tensor_copy(
                                ob_big[:, bh * 1024 + i * CL:
                                       bh * 1024 + (i + 1) * CL],
                                ps_o[i][:])
                        eng = nc.sync if bh % 2 == 0 else nc.scalar
                        eng.dma_start(
                            out=out_d[bh:bh + 1, :],
                            in_=ob_big[:, bh * 1024:(bh + 1) * 1024])

    nc.compile()
    return nc


_NC_CACHE = None


def _get_nc():
    global _NC_CACHE
    if _NC_CACHE is None:
        _NC_CACHE = _build()
    return _NC_CACHE


def make_in_maps(x, Wq, Wk, Wv, Wo):
    import ml_dtypes

    x = np.asarray(x, dtype=np.float32)
    xT = np.ascontiguousarray(x.reshape(BT, C).T)        # [C, BT]
    cosT, sinT = _rope_tables()
    csfull = np.concatenate([cosT, sinT], axis=0)        # [256, T]

    def conv(a):
        return np.ascontiguousarray(a).astype(ml_dtypes.bfloat16)

    def wlay(wT):
        # [C, CL] -> [128, CT*CL] with partition p holding WT[ct*128+p, :]
        return np.ascontiguousarray(
            wT.reshape(CT, 128, CL).transpose(1, 0, 2).reshape(128, CT * CL))

    xb = conv(xT)
    csb = conv(csfull)
    in_maps = []
    for m in range(NCORES):
        sl = slice(m * CL, (m + 1) * CL)
        wall = np.concatenate(
            [conv(wlay(np.asarray(W)[sl, :].T)) for W in (Wq, Wk, Wv, Wo)],
            axis=1)
        in_maps.append({"x": xb, "w": wall, "css": csb})
    return in_maps


def kernel(x, Wq, Wk, Wv, Wo, _trace=False):
    in_maps = make_in_maps(x, Wq, Wk, Wv, Wo)
    nc = _get_nc()
    res = run_bass_kernel_spmd(nc, in_maps, list(range(NCORES)),
                               trace=_trace)
    outs = []
    for m in range(NCORES):
        arr = np.asarray(res.results[m]["out"]).astype(np.float32)
        arr = arr.reshape(4, 128, 4, CL)                 # (bh, p, i, c)
        outs.append(arr.transpose(0, 2, 1, 3).reshape(BT, CL))
    out = np.ascontiguousarray(np.concatenate(outs, axis=1))
    out = out.reshape(B, T, C)
    if _trace:
        return out, res
    return out
